# revision 19
# baseline (speedup 1.0000x reference)
"""Causal self-attention kernel for Trainium2, sharded over 8 NeuronCores.

Problem: B=4, T=2048, DIM=1024, H=16 heads, head_dim=64, fp32 I/O.

Sharding: (batch, head-group) pairs -> 8 shards. Core c handles batch
b = c//2 and head group g = c%2 (8 heads each). Each core computes its
q/k/v projections for its head slice, causal flash-style attention, and
a partial o_proj against its head-slice of wo. The host sums the two
partial o_proj outputs per batch (the "all-reduce") while gathering.

Pipeline strategy (per core): T is processed in 4 chunks of 512. Chunk
c's attention (ACT-exp-bound) is interleaved with chunk c+1's q/k/v
projections and chunk c-1's o_proj (pure PE work) so the tensor engine
never idles long enough for the HAM clock gate to re-throttle it to
1.2 GHz (which is what capped the previous version).

Per-core layout:
  - Host pre-transposes x and the weight slices so the contraction dim
    lands on SBUF partitions, and casts to bf16.
  - Scores are computed TRANSPOSED: sT[tk, tq] = k @ q^T, so softmax'd
    probabilities come out with tk on partitions -- the layout the
    attn@v matmul needs as its moving operand (lhsT = v).
  - The two heads of a pair occupy partitions 0-63 / 64-127 of the same
    QT/KT tile; their scores land in one [128, 1024] psum tile (head A
    cols 0-511, head B cols 512-1023) so ONE scalar-engine exp covers
    both heads (halves ACT instruction count).
  - Softmax skips max-subtraction (scores are O(1) by construction),
    the denominator comes free from a ones column appended to v, and
    1/denom uses the fast DVE reciprocal instead of ACT Ln/Exp.
  - Causal masking inside diagonal 128-tiles: DVE multiply with a
    0/1 lower-triangle mask after the exp.
"""

import numpy as np
import ml_dtypes

import concourse.bass as bass
import concourse.bacc as bacc
import concourse.mybir as mybir
import concourse.tile as tile
from concourse.bass import ds, ts
from concourse.bass_utils import run_bass_kernel_spmd

BF16 = mybir.dt.bfloat16
F32 = mybir.dt.float32

T = 2048
D = 1024
DG = 512          # head-group width (8 heads x 64)
NH = 8            # heads per core
DH = 64
P = 128
NKO = D // P      # 8 contraction tiles for projections
W = 512           # tq chunk width
NCH = T // W      # 4 chunks
NTC = W // P      # 4 t-tiles per chunk
NPAIR = NH // 2   # 4 head pairs

_CACHED = None  # (nc, input names) -- build/trace once per process


def _build_kernel():
    nc = bacc.Bacc("TRN2", target_bir_lowering=False, debug=False)

    # inputs come host-tiled so every DMA reads contiguous HBM
    xT_d = nc.dram_tensor("xT", [NKO, NCH, P, W], BF16, kind="ExternalInput").ap()
    wqT_d = nc.dram_tensor("wqT", [NKO, P, DG], BF16, kind="ExternalInput").ap()
    wkT_d = nc.dram_tensor("wkT", [NKO, P, DG], BF16, kind="ExternalInput").ap()
    wvT_d = nc.dram_tensor("wvT", [NKO, P, DG], BF16, kind="ExternalInput").ap()
    woT_d = nc.dram_tensor("woT", [DG // P, P, D], BF16, kind="ExternalInput").ap()
    y_d = nc.dram_tensor("y", [T, D], F32, kind="ExternalOutput").ap()

    with tile.TileContext(nc) as tc:
        with (
            tc.tile_pool(name="const", bufs=1) as const,
            tc.tile_pool(name="sb", bufs=1) as sb,
            tc.tile_pool(name="work", bufs=4) as work,
            tc.tile_pool(name="wnorm", bufs=2) as wnorm,
            tc.tile_pool(name="ysbp", bufs=4) as ysbp,
            tc.tile_pool(name="ps", bufs=2, space="PSUM") as psp,
            tc.tile_pool(name="av", bufs=2, space="PSUM") as avp,
            tc.tile_pool(name="pj", bufs=2, space="PSUM") as pjp,
        ):
            # ---- constants ----
            # multiplicative causal mask for diag tiles: 1 where tq >= tk
            mskb = const.tile([P, P], BF16, tag="mskb")
            nc.gpsimd.memset(mskb, 1.0)
            nc.gpsimd.affine_select(
                out=mskb, in_=mskb,
                compare_op=mybir.AluOpType.is_ge,
                fill=0.0, base=0,
                pattern=[[1, P]], channel_multiplier=-1,
            )

            # ---- persistent SBUF tensors ----
            XT = sb.tile([P, NKO, T], BF16, tag="XT")
            WQT = sb.tile([P, NKO, DG], BF16, tag="WQT")
            WKT = sb.tile([P, NKO, DG], BF16, tag="WKT")
            WVT = sb.tile([P, NKO, DG], BF16, tag="WVT")
            WOT = sb.tile([P, DG // P, D], BF16, tag="WOT")
            QT = sb.tile([P, DG // P, T], BF16, tag="QT")
            KT = sb.tile([P, DG // P, T], BF16, tag="KT")
            VA = sb.tile([P, T // P, NH, DH + 1], BF16, tag="VA")
            OGT = sb.tile([P, DG // P, T], BF16, tag="OGT")

            # v_aug ones column (before the gpsimd-issued DMAs below)
            nc.gpsimd.memset(VA[:, :, :, DH], 1.0)

            # ---- input DMAs, spread over three issuing engines ----
            # sync: x chunk-0 + wq, with the first k-tile split in quarters
            # across parallel DMA queues so the first projection group can
            # start accumulating within ~2us.
            for q in range(4):
                nc.sync.dma_start(
                    XT[:, 0, ds(q * P, P)], xT_d[0, 0][:, ds(q * P, P)])
            for q in range(4):
                nc.sync.dma_start(
                    WQT[:, 0, ds(q * P, P)], wqT_d[0][:, ds(q * P, P)])
            for k in range(1, NKO):
                nc.sync.dma_start(XT[:, k, 0:W], xT_d[k, 0])
                nc.sync.dma_start(WQT[:, k, :], wqT_d[k])
            # scalar engine issues wk/wv (it is idle until the first exp)
            for k in range(NKO):
                nc.scalar.dma_start(WKT[:, k, :], wkT_d[k])
            for k in range(NKO):
                nc.scalar.dma_start(WVT[:, k, :], wvT_d[k])
            # x chunks 1-3
            for c in range(1, NCH):
                for k in range(NKO):
                    nc.sync.dma_start(XT[:, k, ds(c * W, W)], xT_d[k, c])
            for j in range(DG // P):
                nc.sync.dma_start(WOT[:, j, :], woT_d[j])

            # ---- projection / o_proj emitters (also used as PE filler) ----
            def proj_qk(wsb, dst, c, dg):
                ps = pjp.tile([P, W], F32, tag="pj")
                for k in range(NKO):
                    nc.tensor.matmul(
                        ps,
                        lhsT=wsb[:, k, ts(dg, P)],
                        rhs=XT[:, k, ds(c * W, W)],
                        start=(k == 0), stop=(k == NKO - 1),
                    )
                nc.vector.tensor_copy(dst[:, dg, ds(c * W, W)], ps)

            def proj_v(c, tl):
                tt = c * NTC + tl
                ps = pjp.tile([P, W], F32, tag="pj")
                for k in range(NKO):
                    nc.tensor.matmul(
                        ps,
                        lhsT=XT[:, k, ts(tt, P)],
                        rhs=WVT[:, k, :],
                        start=(k == 0), stop=(k == NKO - 1),
                    )
                nc.vector.tensor_copy(
                    VA[:, tt, :, 0:DH],
                    ps.rearrange("p (h d) -> p h d", h=NH),
                )

            def proj_groups(c):
                gs = []
                for dg in range(DG // P):
                    gs.append(lambda dg=dg: proj_qk(WQT, QT, c, dg))
                for dg in range(DG // P):
                    gs.append(lambda dg=dg: proj_qk(WKT, KT, c, dg))
                for tl in range(NTC):
                    gs.append(lambda tl=tl: proj_v(c, tl))
                return gs

            def oproj_tt(c, tl):
                tt = c * NTC + tl
                ysb = ysbp.tile([P, D], F32, tag="ysb")
                for piece in range(2):
                    ps = pjp.tile([P, W], F32, tag="pj")
                    for jt in range(DG // P):
                        nc.tensor.matmul(
                            ps,
                            lhsT=OGT[:, jt, ts(tt, P)],
                            rhs=WOT[:, jt, ds(piece * W, W)],
                            start=(jt == 0), stop=(jt == DG // P - 1),
                        )
                    nc.vector.tensor_copy(ysb[:, ds(piece * W, W)], ps)
                # split across 4 DMA queues; gpsimd issues them cheaply
                for q in range(4):
                    nc.gpsimd.dma_start(
                        y_d[ts(tt, P), ds(q * 256, 256)],
                        ysb[:, ds(q * 256, 256)],
                    )

            def oproj_groups(c):
                return [lambda tl=tl: oproj_tt(c, tl) for tl in range(NTC)]

            # ---- attention ----
            LAG = 2  # j-iterations of score/exp lookahead before each AV

            def emit_av(pair, avA, avB, item, jmax):
                j, et, off, boff, w = item
                for h, av in ((0, avA), (1, avB)):
                    nc.tensor.matmul(
                        av[0:DH + 1, ds(off, w)],
                        lhsT=VA[:, j, 2 * pair + h, :],
                        rhs=et[:, ds(h * boff, w)],
                        start=(j == 0), stop=(j == jmax),
                    )

            def normalize(av, dst, mul_eng):
                # denominator row straight from psum (off the un chain);
                # the custom-DVE reciprocal is lane-locked, so land it on
                # partition 0 via a plain copy (those may shift base)
                den = wnorm.tile([1, W], F32, tag="den")
                nc.vector.tensor_copy(den, av[DH:DH + 1, :])
                # copy psum out so the av slot frees quickly
                un = wnorm.tile([DH + 1, W], F32, tag="un")
                nc.vector.tensor_copy(un, av[0:DH + 1, :])
                rec = wnorm.tile([1, W], F32, tag="rec")
                nc.vector.reciprocal_approx_fast(rec, den)
                bcb = wnorm.tile([DH, W], F32, tag="bcb")
                nc.gpsimd.partition_broadcast(bcb, rec)
                mul_eng.tensor_mul(dst, un[0:DH, :], bcb)

            def attention_pair(pair, c, pull_filler):
                jmax = (c + 1) * NTC - 1
                avA = avp.tile([P, W], F32, tag="av")
                avB = avp.tile([P, W], F32, tag="av")
                pend = []
                for j in range(jmax + 1):
                    off = max(0, j * P - c * W)
                    w = W - off
                    lo = max(c * W, j * P)
                    diag = j * P >= c * W
                    boff = W
                    ps = psp.tile([P, 2 * W], F32, tag="s")
                    for h in range(2):
                        nc.tensor.matmul(
                            ps[:, ds(h * boff, w)],
                            lhsT=KT[h * DH:(h + 1) * DH, pair, ts(j, P)],
                            rhs=QT[h * DH:(h + 1) * DH, pair, ds(lo, w)],
                            start=True, stop=True,
                        )
                    et = work.tile([P, 2 * W], BF16, tag="et")
                    nc.scalar.activation(
                        et[:, 0:boff + w], ps[:, 0:boff + w],
                        mybir.ActivationFunctionType.Exp,
                        scale=0.125,
                    )
                    if diag:
                        nc.vector.tensor_mul(et[:, 0:P], et[:, 0:P], mskb)
                        nc.vector.tensor_mul(
                            et[:, ds(boff, P)], et[:, ds(boff, P)], mskb)
                    pend.append((j, et, off, boff, w))
                    if len(pend) > LAG:
                        emit_av(pair, avA, avB, pend.pop(0), jmax)
                    pull_filler()
                for item in pend:
                    emit_av(pair, avA, avB, item, jmax)
                # engines support a shifted output partition base: head B's
                # normalized output goes straight into partitions 64-127;
                # B's multiply runs on the idle gpsimd so A/B finish in
                # parallel (matters for the final pair before o_proj)
                normalize(avA, OGT[0:DH, pair, ds(c * W, W)], nc.vector)
                normalize(avB, OGT[DH:P, pair, ds(c * W, W)], nc.gpsimd)

            # ---- main schedule ----
            proj0 = proj_groups(0)
            for g in proj0:
                g()

            for c in range(NCH):
                fillers = []
                if c + 1 < NCH:
                    fillers += proj_groups(c + 1)
                if c >= 1:
                    fillers += oproj_groups(c - 1)
                total_slots = NPAIR * ((c + 1) * NTC)
                state = {"slot": 0, "done": 0}

                def pull_filler():
                    state["slot"] += 1
                    want = len(fillers) * state["slot"] // total_slots
                    while state["done"] < want:
                        fillers[state["done"]]()
                        state["done"] += 1

                for pair in range(NPAIR):
                    attention_pair(pair, c, pull_filler)
                while state["done"] < len(fillers):
                    fillers[state["done"]]()
                    state["done"] += 1

            for g in oproj_groups(NCH - 1):
                g()

    nc.compile()
    return nc


def _get_nc():
    global _CACHED
    if _CACHED is None:
        _CACHED = _build_kernel()
    return _CACHED


def _shard_inputs(x, wq, wk, wv, wo):
    bf = ml_dtypes.bfloat16

    def tile_w(wT):  # [D, DG] -> [NKO, P, DG]
        return np.ascontiguousarray(wT.reshape(NKO, P, DG)).astype(bf)

    in_maps = []
    for core in range(8):
        b, g = divmod(core, 2)
        gs = slice(g * DG, (g + 1) * DG)
        xT = x[b].T  # [D, T]
        x4 = xT.reshape(NKO, P, NCH, W).transpose(0, 2, 1, 3)
        in_maps.append({
            "xT": np.ascontiguousarray(x4).astype(bf),
            "wqT": tile_w(wq[gs, :].T),
            "wkT": tile_w(wk[gs, :].T),
            "wvT": tile_w(wv[gs, :].T),
            "woT": np.ascontiguousarray(
                wo[:, gs].T.reshape(DG // P, P, D)).astype(bf),
        })
    return in_maps


def kernel(x, wq, wk, wv, wo, _trace=False, _trace_cores=None):
    x = np.asarray(x, dtype=np.float32)
    wq = np.asarray(wq, dtype=np.float32)
    wk = np.asarray(wk, dtype=np.float32)
    wv = np.asarray(wv, dtype=np.float32)
    wo = np.asarray(wo, dtype=np.float32)

    nc = _get_nc()
    in_maps = _shard_inputs(x, wq, wk, wv, wo)
    res = run_bass_kernel_spmd(
        nc, in_maps, core_ids=list(range(8)),
        trace=_trace,
        **({"trace_cores": _trace_cores} if _trace_cores else {}),
    )
    B = x.shape[0]
    y = np.zeros((B, T, D), dtype=np.float32)
    for core in range(8):
        b = core // 2
        y[b] += res.results[core]["y"]
    if _trace:
        return y, res
    return y


# revision 22
# speedup vs baseline: 1.3820x; 1.3820x over previous
"""Causal self-attention kernel for Trainium2, sharded over 8 NeuronCores.

Problem: B=4, T=2048, DIM=1024, H=16 heads, head_dim=64, fp32 I/O.

Sharding: (batch, head-group) pairs -> 8 shards. Core c handles batch
b = c//2 and head group g = c%2 (8 heads each). Each core computes its
q/k/v projections for its head slice, causal flash-style attention, and
a partial o_proj against its head-slice of wo. The host sums the two
partial o_proj outputs per batch (the "all-reduce") while gathering.

Pipeline strategy (per core): T is processed in 4 chunks of 512. Chunk
c's attention (ACT-exp-bound) is interleaved with chunk c+1's q/k/v
projections and chunk c-1's o_proj (pure PE work) so the tensor engine
never idles long enough for the HAM clock gate to re-throttle it to
1.2 GHz (which is what capped the previous version).

Per-core layout:
  - Host pre-transposes x and the weight slices so the contraction dim
    lands on SBUF partitions, and casts to bf16.
  - Scores are computed TRANSPOSED: sT[tk, tq] = k @ q^T, so softmax'd
    probabilities come out with tk on partitions -- the layout the
    attn@v matmul needs as its moving operand (lhsT = v).
  - The two heads of a pair occupy partitions 0-63 / 64-127 of the same
    QT/KT tile; their scores land in one [128, 1024] psum tile (head A
    cols 0-511, head B cols 512-1023) so ONE scalar-engine exp covers
    both heads (halves ACT instruction count).
  - Softmax skips max-subtraction (scores are O(1) by construction),
    the denominator comes free from a ones column appended to v, and
    1/denom uses the fast DVE reciprocal instead of ACT Ln/Exp.
  - Causal masking inside diagonal 128-tiles: DVE multiply with a
    0/1 lower-triangle mask after the exp.
"""

import numpy as np
import ml_dtypes

import concourse.bass as bass
import concourse.bacc as bacc
import concourse.mybir as mybir
import concourse.tile as tile
from concourse.bass import ds, ts
from concourse.bass_utils import run_bass_kernel_spmd

BF16 = mybir.dt.bfloat16
F32 = mybir.dt.float32

T = 2048
D = 1024
DG = 512          # head-group width (8 heads x 64)
NH = 8            # heads per core
DH = 64
P = 128
NKO = D // P      # 8 contraction tiles for projections
W = 512           # tq chunk width
NCH = T // W      # 4 chunks
NTC = W // P      # 4 t-tiles per chunk
NPAIR = NH // 2   # 4 head pairs

_CACHED = None  # (nc, input names) -- build/trace once per process


def _build_kernel():
    nc = bacc.Bacc("TRN2", target_bir_lowering=False, debug=False)

    # inputs come host-tiled so every DMA reads contiguous HBM
    xT_d = nc.dram_tensor("xT", [NKO, NCH, P, W], BF16, kind="ExternalInput").ap()
    wqT_d = nc.dram_tensor("wqT", [NKO, P, DG], BF16, kind="ExternalInput").ap()
    wkT_d = nc.dram_tensor("wkT", [NKO, P, DG], BF16, kind="ExternalInput").ap()
    wvT_d = nc.dram_tensor("wvT", [NKO, P, DG], BF16, kind="ExternalInput").ap()
    woT_d = nc.dram_tensor("woT", [DG // P, P, D], BF16, kind="ExternalInput").ap()
    y_d = nc.dram_tensor("y", [T, D], F32, kind="ExternalOutput").ap()

    with tile.TileContext(nc) as tc:
        with (
            tc.tile_pool(name="const", bufs=1) as const,
            tc.tile_pool(name="sb", bufs=1) as sb,
            tc.tile_pool(name="work", bufs=4) as work,
            tc.tile_pool(name="wnorm", bufs=2) as wnorm,
            tc.tile_pool(name="ysbp", bufs=6) as ysbp,
            tc.tile_pool(name="ps", bufs=2, space="PSUM") as psp,
            tc.tile_pool(name="av", bufs=2, space="PSUM") as avp,
            tc.tile_pool(name="pj", bufs=2, space="PSUM") as pjp,
        ):
            # ---- constants ----
            # multiplicative causal mask for diag tiles: 1 where tq >= tk
            mskb = const.tile([P, P], BF16, tag="mskb")
            nc.gpsimd.memset(mskb, 1.0)
            nc.gpsimd.affine_select(
                out=mskb, in_=mskb,
                compare_op=mybir.AluOpType.is_ge,
                fill=0.0, base=0,
                pattern=[[1, P]], channel_multiplier=-1,
            )

            # ---- persistent SBUF tensors ----
            XT = sb.tile([P, NKO, T], BF16, tag="XT")
            WQT = sb.tile([P, NKO, DG], BF16, tag="WQT")
            WKT = sb.tile([P, NKO, DG], BF16, tag="WKT")
            WVT = sb.tile([P, NKO, DG], BF16, tag="WVT")
            WOT = sb.tile([P, DG // P, D], BF16, tag="WOT")
            QT = sb.tile([P, DG // P, T], BF16, tag="QT")
            KT = sb.tile([P, DG // P, T], BF16, tag="KT")
            VA = sb.tile([P, T // P, NH, DH + 1], BF16, tag="VA")
            OGT = sb.tile([P, DG // P, T], BF16, tag="OGT")

            # v_aug ones column (before the gpsimd-issued DMAs below)
            nc.gpsimd.memset(VA[:, :, :, DH], 1.0)

            # ---- input DMAs, spread over three issuing engines ----
            # sync: x chunk-0 + wq, with the first k-tile split in quarters
            # across parallel DMA queues so the first projection group can
            # start accumulating within ~2us.
            for q in range(4):
                nc.sync.dma_start(
                    XT[:, 0, ds(q * P, P)], xT_d[0, 0][:, ds(q * P, P)])
            for q in range(4):
                nc.sync.dma_start(
                    WQT[:, 0, ds(q * P, P)], wqT_d[0][:, ds(q * P, P)])
            for k in range(1, NKO):
                nc.sync.dma_start(XT[:, k, 0:W], xT_d[k, 0])
                nc.sync.dma_start(WQT[:, k, :], wqT_d[k])
            # scalar engine issues wk/wv (it is idle until the first exp)
            for k in range(NKO):
                nc.scalar.dma_start(WKT[:, k, :], wkT_d[k])
            for k in range(NKO):
                nc.scalar.dma_start(WVT[:, k, :], wvT_d[k])
            # x chunks 1-3
            for c in range(1, NCH):
                for k in range(NKO):
                    nc.sync.dma_start(XT[:, k, ds(c * W, W)], xT_d[k, c])
            for j in range(DG // P):
                nc.sync.dma_start(WOT[:, j, :], woT_d[j])

            # ---- projection / o_proj emitters (also used as PE filler) ----
            def proj_qk(wsb, dst, c, dg):
                ps = pjp.tile([P, W], F32, tag="pj")
                for k in range(NKO):
                    nc.tensor.matmul(
                        ps,
                        lhsT=wsb[:, k, ts(dg, P)],
                        rhs=XT[:, k, ds(c * W, W)],
                        start=(k == 0), stop=(k == NKO - 1),
                    )
                nc.vector.tensor_copy(dst[:, dg, ds(c * W, W)], ps)

            def proj_v(c, tl):
                tt = c * NTC + tl
                ps = pjp.tile([P, W], F32, tag="pj")
                for k in range(NKO):
                    nc.tensor.matmul(
                        ps,
                        lhsT=XT[:, k, ts(tt, P)],
                        rhs=WVT[:, k, :],
                        start=(k == 0), stop=(k == NKO - 1),
                    )
                nc.vector.tensor_copy(
                    VA[:, tt, :, 0:DH],
                    ps.rearrange("p (h d) -> p h d", h=NH),
                )

            def proj_groups(c):
                gs = []
                for dg in range(DG // P):
                    gs.append(lambda dg=dg: proj_qk(WQT, QT, c, dg))
                for dg in range(DG // P):
                    gs.append(lambda dg=dg: proj_qk(WKT, KT, c, dg))
                for tl in range(NTC):
                    gs.append(lambda tl=tl: proj_v(c, tl))
                return gs

            def oproj_tt(c, tl):
                tt = c * NTC + tl
                ysb = ysbp.tile([P, D], F32, tag="ysb")
                for piece in range(2):
                    ps = pjp.tile([P, W], F32, tag="pj")
                    for jt in range(DG // P):
                        nc.tensor.matmul(
                            ps,
                            lhsT=OGT[:, jt, ts(tt, P)],
                            rhs=WOT[:, jt, ds(piece * W, W)],
                            start=(jt == 0), stop=(jt == DG // P - 1),
                        )
                    nc.vector.tensor_copy(ysb[:, ds(piece * W, W)], ps)
                # split across 4 DMA queues so the transfer (which gates
                # ysb buffer reuse) finishes in ~5us instead of ~20us
                for q in range(4):
                    nc.sync.dma_start(
                        y_d[ts(tt, P), ds(q * 256, 256)],
                        ysb[:, ds(q * 256, 256)],
                    )

            def oproj_groups(c):
                return [lambda tl=tl: oproj_tt(c, tl) for tl in range(NTC)]

            # ---- attention ----
            LAG = 2  # j-iterations of score/exp lookahead before each AV

            def emit_av(pair, avA, avB, item, jmax):
                j, et, off, boff, w = item
                for h, av in ((0, avA), (1, avB)):
                    nc.tensor.matmul(
                        av[0:DH + 1, ds(off, w)],
                        lhsT=VA[:, j, 2 * pair + h, :],
                        rhs=et[:, ds(h * boff, w)],
                        start=(j == 0), stop=(j == jmax),
                    )

            def normalize(av, dst, mul_eng):
                # denominator row straight from psum (off the un chain);
                # the custom-DVE reciprocal is lane-locked, so land it on
                # partition 0 via a plain copy (those may shift base)
                den = wnorm.tile([1, W], F32, tag="den")
                nc.vector.tensor_copy(den, av[DH:DH + 1, :])
                # copy psum out so the av slot frees quickly
                un = wnorm.tile([DH + 1, W], F32, tag="un")
                nc.vector.tensor_copy(un, av[0:DH + 1, :])
                rec = wnorm.tile([1, W], F32, tag="rec")
                nc.vector.reciprocal_approx_fast(rec, den)
                bcb = wnorm.tile([DH, W], F32, tag="bcb")
                nc.gpsimd.partition_broadcast(bcb, rec)
                mul_eng.tensor_mul(dst, un[0:DH, :], bcb)

            def attention_pair(pair, c, pull_filler):
                jmax = (c + 1) * NTC - 1
                avA = avp.tile([P, W], F32, tag="av")
                avB = avp.tile([P, W], F32, tag="av")
                pend = []
                for j in range(jmax + 1):
                    off = max(0, j * P - c * W)
                    w = W - off
                    lo = max(c * W, j * P)
                    diag = j * P >= c * W
                    boff = W
                    ps = psp.tile([P, 2 * W], F32, tag="s")
                    for h in range(2):
                        nc.tensor.matmul(
                            ps[:, ds(h * boff, w)],
                            lhsT=KT[h * DH:(h + 1) * DH, pair, ts(j, P)],
                            rhs=QT[h * DH:(h + 1) * DH, pair, ds(lo, w)],
                            start=True, stop=True,
                        )
                    et = work.tile([P, 2 * W], BF16, tag="et")
                    nc.scalar.activation(
                        et[:, 0:boff + w], ps[:, 0:boff + w],
                        mybir.ActivationFunctionType.Exp,
                        scale=0.125,
                    )
                    if diag:
                        nc.vector.tensor_mul(et[:, 0:P], et[:, 0:P], mskb)
                        nc.vector.tensor_mul(
                            et[:, ds(boff, P)], et[:, ds(boff, P)], mskb)
                    pend.append((j, et, off, boff, w))
                    if len(pend) > LAG:
                        emit_av(pair, avA, avB, pend.pop(0), jmax)
                    pull_filler()
                for item in pend:
                    emit_av(pair, avA, avB, item, jmax)
                # engines support a shifted output partition base: head B's
                # normalized output goes straight into partitions 64-127;
                # B's multiply runs on the idle gpsimd so A/B finish in
                # parallel (matters for the final pair before o_proj)
                normalize(avA, OGT[0:DH, pair, ds(c * W, W)], nc.vector)
                normalize(avB, OGT[DH:P, pair, ds(c * W, W)], nc.vector)

            # ---- main schedule ----
            proj0 = proj_groups(0)
            for g in proj0:
                g()

            for c in range(NCH):
                fillers = []
                if c + 1 < NCH:
                    fillers += proj_groups(c + 1)
                if c >= 1:
                    fillers += oproj_groups(c - 1)
                total_slots = NPAIR * ((c + 1) * NTC)
                state = {"slot": 0, "done": 0}

                def pull_filler():
                    state["slot"] += 1
                    want = len(fillers) * state["slot"] // total_slots
                    while state["done"] < want:
                        fillers[state["done"]]()
                        state["done"] += 1

                for pair in range(NPAIR):
                    attention_pair(pair, c, pull_filler)
                while state["done"] < len(fillers):
                    fillers[state["done"]]()
                    state["done"] += 1

            for g in oproj_groups(NCH - 1):
                g()

    nc.compile()
    return nc


def _get_nc():
    global _CACHED
    if _CACHED is None:
        _CACHED = _build_kernel()
    return _CACHED


def _shard_inputs(x, wq, wk, wv, wo):
    bf = ml_dtypes.bfloat16

    def tile_w(wT):  # [D, DG] -> [NKO, P, DG]
        return np.ascontiguousarray(wT.reshape(NKO, P, DG)).astype(bf)

    in_maps = []
    for core in range(8):
        b, g = divmod(core, 2)
        gs = slice(g * DG, (g + 1) * DG)
        xT = x[b].T  # [D, T]
        x4 = xT.reshape(NKO, P, NCH, W).transpose(0, 2, 1, 3)
        in_maps.append({
            "xT": np.ascontiguousarray(x4).astype(bf),
            "wqT": tile_w(wq[gs, :].T),
            "wkT": tile_w(wk[gs, :].T),
            "wvT": tile_w(wv[gs, :].T),
            "woT": np.ascontiguousarray(
                wo[:, gs].T.reshape(DG // P, P, D)).astype(bf),
        })
    return in_maps


def kernel(x, wq, wk, wv, wo, _trace=False, _trace_cores=None):
    x = np.asarray(x, dtype=np.float32)
    wq = np.asarray(wq, dtype=np.float32)
    wk = np.asarray(wk, dtype=np.float32)
    wv = np.asarray(wv, dtype=np.float32)
    wo = np.asarray(wo, dtype=np.float32)

    nc = _get_nc()
    in_maps = _shard_inputs(x, wq, wk, wv, wo)
    res = run_bass_kernel_spmd(
        nc, in_maps, core_ids=list(range(8)),
        trace=_trace,
        **({"trace_cores": _trace_cores} if _trace_cores else {}),
    )
    B = x.shape[0]
    y = np.zeros((B, T, D), dtype=np.float32)
    for core in range(8):
        b = core // 2
        y[b] += res.results[core]["y"]
    if _trace:
        return y, res
    return y


# revision 27
# speedup vs baseline: 1.3980x; 1.0116x over previous
"""Causal self-attention kernel for Trainium2, sharded over 8 NeuronCores.

Problem: B=4, T=2048, DIM=1024, H=16 heads, head_dim=64, fp32 I/O.

Sharding: (batch, head-group) pairs -> 8 shards. Core c handles batch
b = c//2 and head group g = c%2 (8 heads each). Each core computes its
q/k/v projections for its head slice, causal flash-style attention, and
a partial o_proj against its head-slice of wo. The host sums the two
partial o_proj outputs per batch (the "all-reduce") while gathering.

Pipeline strategy (per core): T is processed in 4 chunks of 512. Chunk
c's attention (ACT-exp-bound) is interleaved with chunk c+1's q/k/v
projections and chunk c-1's o_proj (pure PE work) so the tensor engine
never idles long enough for the HAM clock gate to re-throttle it to
1.2 GHz (which is what capped the previous version).

Per-core layout:
  - Host pre-transposes x and the weight slices so the contraction dim
    lands on SBUF partitions, and casts to bf16.
  - Scores are computed TRANSPOSED: sT[tk, tq] = k @ q^T, so softmax'd
    probabilities come out with tk on partitions -- the layout the
    attn@v matmul needs as its moving operand (lhsT = v).
  - The two heads of a pair occupy partitions 0-63 / 64-127 of the same
    QT/KT tile; their scores land in one [128, 1024] psum tile (head A
    cols 0-511, head B cols 512-1023) so ONE scalar-engine exp covers
    both heads (halves ACT instruction count).
  - Softmax skips max-subtraction (scores are O(1) by construction),
    the denominator comes free from a ones column appended to v, and
    1/denom uses the fast DVE reciprocal instead of ACT Ln/Exp.
  - Causal masking inside diagonal 128-tiles: DVE multiply with a
    0/1 lower-triangle mask after the exp.
"""

import numpy as np
import ml_dtypes

import concourse.bass as bass
import concourse.bacc as bacc
import concourse.mybir as mybir
import concourse.tile as tile
from concourse.bass import ds, ts
from concourse.bass_utils import run_bass_kernel_spmd

BF16 = mybir.dt.bfloat16
F32 = mybir.dt.float32

T = 2048
D = 1024
DG = 512          # head-group width (8 heads x 64)
NH = 8            # heads per core
DH = 64
P = 128
NKO = D // P      # 8 contraction tiles for projections
W = 512           # tq chunk width
NCH = T // W      # 4 chunks
NTC = W // P      # 4 t-tiles per chunk
NPAIR = NH // 2   # 4 head pairs

_CACHED = None  # (nc, input names) -- build/trace once per process


def _build_kernel():
    nc = bacc.Bacc("TRN2", target_bir_lowering=False, debug=False)

    # inputs come host-tiled so every DMA reads contiguous HBM
    xT_d = nc.dram_tensor("xT", [NKO, NCH, P, W], BF16, kind="ExternalInput").ap()
    wqT_d = nc.dram_tensor("wqT", [NKO, P, DG], BF16, kind="ExternalInput").ap()
    wkT_d = nc.dram_tensor("wkT", [NKO, P, DG], BF16, kind="ExternalInput").ap()
    wvT_d = nc.dram_tensor("wvT", [NKO, P, DG], BF16, kind="ExternalInput").ap()
    woT_d = nc.dram_tensor("woT", [DG // P, P, D], BF16, kind="ExternalInput").ap()
    y_d = nc.dram_tensor("y", [T, D], F32, kind="ExternalOutput").ap()

    with tile.TileContext(nc) as tc:
        with (
            tc.tile_pool(name="const", bufs=1) as const,
            tc.tile_pool(name="sb", bufs=1) as sb,
            tc.tile_pool(name="work", bufs=4) as work,
            tc.tile_pool(name="wnorm", bufs=2) as wnorm,
            tc.tile_pool(name="ysbp", bufs=6) as ysbp,
            tc.tile_pool(name="ps", bufs=2, space="PSUM") as psp,
            tc.tile_pool(name="av", bufs=2, space="PSUM") as avp,
            tc.tile_pool(name="pj", bufs=2, space="PSUM") as pjp,
        ):
            # ---- constants ----
            # multiplicative causal mask for diag tiles: 1 where tq >= tk
            mskb = const.tile([P, P], BF16, tag="mskb")
            nc.gpsimd.memset(mskb, 1.0)
            nc.gpsimd.affine_select(
                out=mskb, in_=mskb,
                compare_op=mybir.AluOpType.is_ge,
                fill=0.0, base=0,
                pattern=[[1, P]], channel_multiplier=-1,
            )

            # ---- persistent SBUF tensors ----
            XT = sb.tile([P, NKO, T], BF16, tag="XT")
            WQT = sb.tile([P, NKO, DG], BF16, tag="WQT")
            WKT = sb.tile([P, NKO, DG], BF16, tag="WKT")
            WVT = sb.tile([P, NKO, DG], BF16, tag="WVT")
            WOT = sb.tile([P, DG // P, D], BF16, tag="WOT")
            QT = sb.tile([P, DG // P, T], BF16, tag="QT")
            KT = sb.tile([P, DG // P, T], BF16, tag="KT")
            VA = sb.tile([P, T // P, NH, DH + 1], BF16, tag="VA")
            # one tile per head pair: o_proj's jt-accumulation then only
            # waits on the pair it actually reads
            OGT = [
                sb.tile([P, T], BF16, name=f"OGT{p}", tag=f"OGT{p}")
                for p in range(NPAIR)
            ]

            # v_aug ones column (before the gpsimd-issued DMAs below)
            nc.gpsimd.memset(VA[:, :, :, DH], 1.0)

            # ---- input DMAs, spread over three issuing engines ----
            # sync: x chunk-0 + wq, with the first k-tile split in quarters
            # across parallel DMA queues so the first projection group can
            # start accumulating within ~2us.
            for k in range(NKO):
                nc.sync.dma_start(XT[:, k, 0:W], xT_d[k, 0])
                nc.sync.dma_start(WQT[:, k, :], wqT_d[k])
            # scalar engine issues wk/wv (it is idle until the first exp)
            for k in range(NKO):
                nc.scalar.dma_start(WKT[:, k, :], wkT_d[k])
            for k in range(NKO):
                nc.scalar.dma_start(WVT[:, k, :], wvT_d[k])
            # x chunks 1-3
            for c in range(1, NCH):
                for k in range(NKO):
                    nc.sync.dma_start(XT[:, k, ds(c * W, W)], xT_d[k, c])
            for j in range(DG // P):
                nc.sync.dma_start(WOT[:, j, :], woT_d[j])

            # ---- projection / o_proj emitters (also used as PE filler) ----
            def proj_qk(wsb, dst, c, dg):
                ps = pjp.tile([P, W], F32, tag="pj")
                for k in range(NKO):
                    nc.tensor.matmul(
                        ps,
                        lhsT=wsb[:, k, ts(dg, P)],
                        rhs=XT[:, k, ds(c * W, W)],
                        start=(k == 0), stop=(k == NKO - 1),
                    )
                nc.vector.tensor_copy(dst[:, dg, ds(c * W, W)], ps)

            def proj_v(c, tl):
                tt = c * NTC + tl
                ps = pjp.tile([P, W], F32, tag="pj")
                for k in range(NKO):
                    nc.tensor.matmul(
                        ps,
                        lhsT=XT[:, k, ts(tt, P)],
                        rhs=WVT[:, k, :],
                        start=(k == 0), stop=(k == NKO - 1),
                    )
                nc.vector.tensor_copy(
                    VA[:, tt, :, 0:DH],
                    ps.rearrange("p (h d) -> p h d", h=NH),
                )

            def proj_groups(c):
                gs = []
                for dg in range(DG // P):
                    gs.append(lambda dg=dg: proj_qk(WQT, QT, c, dg))
                for dg in range(DG // P):
                    gs.append(lambda dg=dg: proj_qk(WKT, KT, c, dg))
                for tl in range(NTC):
                    gs.append(lambda tl=tl: proj_v(c, tl))
                return gs

            def oproj_tt(c, tl):
                tt = c * NTC + tl
                ysb = ysbp.tile([P, D], F32, tag="ysb")
                for piece in range(2):
                    ps = pjp.tile([P, W], F32, tag="pj")
                    for jt in range(DG // P):
                        nc.tensor.matmul(
                            ps,
                            lhsT=OGT[jt][:, ts(tt, P)],
                            rhs=WOT[:, jt, ds(piece * W, W)],
                            start=(jt == 0), stop=(jt == DG // P - 1),
                        )
                    nc.vector.tensor_copy(ysb[:, ds(piece * W, W)], ps)
                # split across 4 DMA queues so the transfer (which gates
                # ysb buffer reuse) finishes in ~5us instead of ~20us
                for q in range(4):
                    nc.sync.dma_start(
                        y_d[ts(tt, P), ds(q * 256, 256)],
                        ysb[:, ds(q * 256, 256)],
                    )

            def oproj_groups(c):
                return [lambda tl=tl: oproj_tt(c, tl) for tl in range(NTC)]

            # ---- attention ----
            LAG = 2  # j-iterations of score/exp lookahead before each AV

            def emit_av(pair, avA, avB, item, jmax):
                j, et, off, boff, w = item
                for h, av in ((0, avA), (1, avB)):
                    nc.tensor.matmul(
                        av[0:DH + 1, ds(off, w)],
                        lhsT=VA[:, j, 2 * pair + h, :],
                        rhs=et[:, ds(h * boff, w)],
                        start=(j == 0), stop=(j == jmax),
                    )

            def normalize(av, dst, mul_eng):
                # denominator row straight from psum (off the un chain);
                # the custom-DVE reciprocal is lane-locked, so land it on
                # partition 0 via a plain copy (those may shift base)
                den = wnorm.tile([1, W], F32, tag="den")
                nc.vector.tensor_copy(den, av[DH:DH + 1, :])
                # copy psum out so the av slot frees quickly
                un = wnorm.tile([DH + 1, W], F32, tag="un")
                nc.vector.tensor_copy(un, av[0:DH + 1, :])
                rec = wnorm.tile([1, W], F32, tag="rec")
                nc.vector.reciprocal_approx_fast(rec, den)
                bcb = wnorm.tile([DH, W], F32, tag="bcb")
                nc.gpsimd.partition_broadcast(bcb, rec)
                mul_eng.tensor_mul(dst, un[0:DH, :], bcb)

            def attention_pair(pair, c, pull_filler):
                jmax = (c + 1) * NTC - 1
                avA = avp.tile([P, W], F32, tag="av")
                avB = avp.tile([P, W], F32, tag="av")
                pend = []
                for j in range(jmax + 1):
                    off = max(0, j * P - c * W)
                    w = W - off
                    lo = max(c * W, j * P)
                    diag = j * P >= c * W
                    boff = W
                    ps = psp.tile([P, 2 * W], F32, tag="s")
                    for h in range(2):
                        nc.tensor.matmul(
                            ps[:, ds(h * boff, w)],
                            lhsT=KT[h * DH:(h + 1) * DH, pair, ts(j, P)],
                            rhs=QT[h * DH:(h + 1) * DH, pair, ds(lo, w)],
                            start=True, stop=True,
                        )
                    et = work.tile([P, 2 * W], BF16, tag="et")
                    nc.scalar.activation(
                        et[:, 0:boff + w], ps[:, 0:boff + w],
                        mybir.ActivationFunctionType.Exp,
                        scale=0.125,
                    )
                    if diag:
                        nc.vector.tensor_mul(et[:, 0:P], et[:, 0:P], mskb)
                        nc.vector.tensor_mul(
                            et[:, ds(boff, P)], et[:, ds(boff, P)], mskb)
                    pend.append((j, et, off, boff, w))
                    if len(pend) > LAG:
                        emit_av(pair, avA, avB, pend.pop(0), jmax)
                    pull_filler()
                for item in pend:
                    emit_av(pair, avA, avB, item, jmax)
                # engines support a shifted output partition base: head B's
                # normalized output goes straight into partitions 64-127;
                # B's multiply runs on the idle gpsimd so A/B finish in
                # parallel (matters for the final pair before o_proj)
                normalize(avA, OGT[pair][0:DH, ds(c * W, W)], nc.vector)
                normalize(avB, OGT[pair][DH:P, ds(c * W, W)], nc.vector)

            # ---- main schedule ----
            proj0 = proj_groups(0)
            for g in proj0:
                g()

            for c in range(NCH):
                fillers = []
                if c + 1 < NCH:
                    fillers += proj_groups(c + 1)
                if c >= 1:
                    fillers += oproj_groups(c - 1)
                total_slots = NPAIR * ((c + 1) * NTC)
                state = {"slot": 0, "done": 0}

                def pull_filler():
                    state["slot"] += 1
                    want = len(fillers) * state["slot"] // total_slots
                    while state["done"] < want:
                        fillers[state["done"]]()
                        state["done"] += 1

                for pair in range(NPAIR):
                    attention_pair(pair, c, pull_filler)
                while state["done"] < len(fillers):
                    fillers[state["done"]]()
                    state["done"] += 1

            for g in oproj_groups(NCH - 1):
                g()

    nc.compile()
    return nc


def _get_nc():
    global _CACHED
    if _CACHED is None:
        _CACHED = _build_kernel()
    return _CACHED


def _shard_inputs(x, wq, wk, wv, wo):
    bf = ml_dtypes.bfloat16

    def tile_w(wT):  # [D, DG] -> [NKO, P, DG]
        return np.ascontiguousarray(wT.reshape(NKO, P, DG)).astype(bf)

    in_maps = []
    for core in range(8):
        b, g = divmod(core, 2)
        gs = slice(g * DG, (g + 1) * DG)
        xT = x[b].T  # [D, T]
        x4 = xT.reshape(NKO, P, NCH, W).transpose(0, 2, 1, 3)
        in_maps.append({
            "xT": np.ascontiguousarray(x4).astype(bf),
            "wqT": tile_w(wq[gs, :].T),
            "wkT": tile_w(wk[gs, :].T),
            "wvT": tile_w(wv[gs, :].T),
            "woT": np.ascontiguousarray(
                wo[:, gs].T.reshape(DG // P, P, D)).astype(bf),
        })
    return in_maps


def kernel(x, wq, wk, wv, wo, _trace=False, _trace_cores=None):
    x = np.asarray(x, dtype=np.float32)
    wq = np.asarray(wq, dtype=np.float32)
    wk = np.asarray(wk, dtype=np.float32)
    wv = np.asarray(wv, dtype=np.float32)
    wo = np.asarray(wo, dtype=np.float32)

    nc = _get_nc()
    in_maps = _shard_inputs(x, wq, wk, wv, wo)
    res = run_bass_kernel_spmd(
        nc, in_maps, core_ids=list(range(8)),
        trace=_trace,
        **({"trace_cores": _trace_cores} if _trace_cores else {}),
    )
    B = x.shape[0]
    y = np.zeros((B, T, D), dtype=np.float32)
    for core in range(8):
        b = core // 2
        y[b] += res.results[core]["y"]
    if _trace:
        return y, res
    return y
